# revision 19
# baseline (speedup 1.0000x reference)
"""Trainium2 Bass kernel for nn_Expander (broadcast -> Conv3d(3->4) -> Conv3d(4->3)).

Math: the conv input is x (B,3) broadcast over all spatial positions, so the
whole network is an affine map per batch row:  out[b] = x[b] @ M + K0.
With two stacked kernel-3 SAME convs, out positions only depend on their
distance-from-edge class per axis: classes {0, 1, interior, n-2, n-1}.

Host side: fold (w1,b1,w2,b2) into W (4, 3*5*5*28) bf16 via a 4-row numpy
probe (3 basis rows + zero row), with the w-axis pre-expanded to 28 so the
device never does a w-expansion.  Column order per channel p is
[cd2, cd0, cd1] then [cd3, cd4] so the interior (cd2) block lands first.

Device side (per core, 128 batch rows).  The output write stream caps at
~429 GB/s (measured HBM-write ceiling) = 45 us for 19.3 MB, so everything
else is organized to start that stream as early as possible and keep it
gapless:
  1. one input DMA: [x_aug^T (4,128) | W (4,2100)] bf16
  2. bf16 matmuls (single pass, no fp32 LOW/HIGH double-pump); p0 is
     split 140/280/280 cols so the teaser chain waits only the 140-col
     cd2 matmul; p1/p2 use 420/280                       [TensorE]
  3. h-expand 5 -> 28 straight from PSUM:
       interior plane (cd2) + 3 replicas in a 4-plane rep tile  -> DVE
       d-edge planes in one [lohi][p][plane] tile               -> ACT
     Every tile has exactly one writer engine and is fully written before
     any DMA reads it -- no cross-engine WAW/WAR dependences possible.
  4. DMA out (nc.sync, FIFO): teaser d2:6 from the single interior plane
     (broadcast source; starts the stream ~2.5 us after input arrives),
     then PLAIN re-reads of each rep tile for d6:10/d10:14 (and d2:6 for
     p1/p2) at 12544B units (~430 GB/s; broadcast units lose 2.5-7%),
     then two merged edge DMAs covering all p at once.
Measured: ~61.9 us on healthy runs.  ~35-40% of runs see a stochastic
cross-core HBM slowdown (often SDMA engine 15 alone at ~21 GB/s ->
backlog trickle tail, +8-10 us); probe kernels show the same effect with
pure DMA streams, so it is environmental, not kernel-induced.
"""

import numpy as np
import ml_dtypes

import concourse.bass as bass
import concourse.mybir as mybir
from concourse.tile import TileContext
from concourse.bass_utils import run_bass_kernel_spmd


def _ensure_axon_hooks_stub():
    """concourse imports antenv.axon_hooks when BASS_TRACE=1 under axon; the
    module is absent on this image.  Provide a no-op stub (profiling then
    degrades gracefully) unless a real one is already installed."""
    import sys, types

    try:
        import antenv.axon_hooks  # noqa: F401
    except ImportError:
        import antenv

        mod = types.ModuleType("antenv.axon_hooks")
        mod._hook = None
        mod.set_axon_ntff_profile_hook = lambda h: setattr(mod, "_hook", h)
        mod.get_axon_ntff_profile_hook = lambda: mod._hook
        sys.modules["antenv.axon_hooks"] = mod
        antenv.axon_hooks = mod


_ensure_axon_hooks_stub()


def _split_multi_waits(nc):
    """This container's walrus accepts at most ONE sync-wait (and update)
    command per instruction.  Tile can attach several (e.g. the kernel-tail
    Drain waits per outstanding semaphore; DMAs get cross-lane WAW waits).
    Hoist the extras onto injected same-engine NoOps: waits go on NoOps
    placed immediately BEFORE the instruction (waiting earlier on the same
    queue is equivalent), extra updates on NoOps AFTER it."""
    uid = [0]
    for f in nc.m.functions:
        for bb in f.blocks:
            out = []
            changed = False
            for inst in bb.instructions:
                si = getattr(inst, "sync_info", None)
                ow = list(si.on_wait) if si is not None and si.on_wait else []
                ou = list(si.on_update) if si is not None and si.on_update else []
                pre, post = [], []
                if len(ow) > 1 or len(ou) > 1:
                    def mknop(w=None, u=None):
                        uid[0] += 1
                        nop = mybir.InstNoOp(
                            name=f"{inst.name}-sw{uid[0]}",
                            opcode="NoOp",
                            engine=inst.engine,
                            debug=inst.debug,
                            ins=[],
                            outs=[],
                        )
                        nop.sync_info = mybir.SyncInfo(
                            on_wait=[w] if w else [], on_update=[u] if u else []
                        )
                        return nop

                    pre = [mknop(w=w) for w in ow[:-1]]
                    post = [mknop(u=u) for u in ou[1:]]
                    inst.sync_info = mybir.SyncInfo(
                        on_wait=ow[-1:], on_update=ou[:1]
                    )
                    changed = True
                out.extend(pre)
                out.append(inst)
                out.extend(post)
            if changed:
                bb.instructions = out


B, C, F, S = 1024, 3, 16, 28
P_OUT = 3
N_CORES = 8
BL = B // N_CORES  # 128 batch rows per core
NCLS = 5  # position classes per spatial axis
SS = S * S  # 784
NWC = NCLS * S  # 140 cols per (p, cd): (ch, w) with w expanded
NJ = P_OUT * NCLS * NWC  # 2100 weight columns
F32 = mybir.dt.float32
BF16 = mybir.dt.bfloat16
# per-p column blocks: a = [cd2, cd0, cd1] (420 cols), b = [cd3, cd4] (280)
CDA = [2, 0, 1]
CDB = [3, 4]
NA = len(CDA) * NWC  # 420
NB = len(CDB) * NWC  # 280


def _conv3d_same(x, w):
    """x (B,Ci,D,H,W), w (Co,Ci,3,3,3) -> (B,Co,D,H,W), SAME padding."""
    Bp, Ci, D, H, W = x.shape
    xp = np.pad(x, ((0, 0), (0, 0), (1, 1), (1, 1), (1, 1)))
    out = np.zeros((Bp, w.shape[0], D, H, W), x.dtype)
    for kd in range(3):
        for kh in range(3):
            for kw in range(3):
                out += np.einsum(
                    "oc,bcdhw->bodhw",
                    w[:, :, kd, kh, kw],
                    xp[:, :, kd : kd + D, kh : kh + H, kw : kw + W],
                )
    return out


def _fold_weights(w1, b1, w2, b2):
    """Return W (4, 2100) bf16: rows 0..2 = linear response to e_c, row 3 =
    constant term, at the 5(d-class) x 5(h-class) x 28(w, full) output
    representatives, column order per p: cd in [2,0,1,3,4]."""
    probe = np.zeros((4, C), np.float64)
    probe[:3] = np.eye(C)
    vp = np.broadcast_to(probe[:, :, None, None, None], (4, C, F, S, S)).astype(
        np.float64
    )
    y = _conv3d_same(vp, w1.astype(np.float64))
    y += b1.astype(np.float64)[None, :, None, None, None]
    y = _conv3d_same(y, w2.astype(np.float64))
    y += b2.astype(np.float64)[None, :, None, None, None]
    k0 = y[3]  # (3,16,28,28) constant part
    m = y[:3] - k0[None]  # (3,3,16,28,28) linear part

    dr = [0, 1, 2, F - 2, F - 1]
    hr = [0, 1, 2, S - 2, S - 1]
    # reps: (row, p, cd, ch, w) with cd/ch classed, w full
    mreps = m[:, :, dr][:, :, :, hr]  # (3, 3, 5, 5, 28)
    kreps = k0[:, dr][:, :, hr]  # (3, 5, 5, 28)
    w_all = np.empty((4, P_OUT, NCLS, NCLS, S), np.float64)
    w_all[:3] = mreps
    w_all[3] = kreps
    # reorder cd to [2,0,1,3,4] and flatten to (4, 2100)
    w_all = w_all[:, :, CDA + CDB]
    return np.ascontiguousarray(
        w_all.reshape(4, NJ).astype(ml_dtypes.bfloat16)
    )


def _build_bass():
    nc = bass.Bass()
    # packed input: cols [0:BL] = x_aug^T (4,128), cols [BL:] = W (4,2100)
    xw = nc.dram_tensor("xw", [4, BL + NJ], BF16, kind="ExternalInput")
    out = nc.dram_tensor("out", [BL, P_OUT, F, S, S], F32, kind="ExternalOutput")
    out_v = out[:].rearrange("b p d h w -> b p d (h w)")  # (128, 3, 16, 784)

    with TileContext(nc) as tc:
        with (
            tc.tile_pool(name="pool", bufs=1) as pool,
            tc.tile_pool(name="psum", bufs=1, space="PSUM") as psum_pool,
        ):
            xw_sb = pool.tile([4, BL + NJ], BF16)
            nc.sync.dma_start(out=xw_sb[:], in_=xw[:])

            # one writer engine per tile; every tile fully written before read
            ifirst0 = pool.tile([BL, 1, S, S], F32)  # DVE; p0 teaser source
            rep = [
                pool.tile([BL, 4, SS], F32, name=f"rep{i}")  # DVE; 4 I-planes
                for i in range(P_OUT)
            ]
            # d-edge planes, ACT-written, laid out [lohi][p][plane] so each
            # of the two edge DMAs reads one contiguous 18816B/partition run
            edge = pool.tile([BL, 2, P_OUT, 2, S, S], F32)
            # p0 gets a 3-block matmul split (cd2 alone first, so the teaser
            # chain waits only a 140-col matmul); p1/p2 use 420+280 blocks
            ps_a0 = psum_pool.tile([BL, NWC], F32)
            ps_b0 = psum_pool.tile([BL, 2 * NWC], F32)
            ps_c0 = psum_pool.tile([BL, NB], F32)
            ps_a = [None] + [
                psum_pool.tile([BL, NA], F32, name=f"ps_a{i}") for i in (1, 2)
            ]
            ps_b = [None] + [
                psum_pool.tile([BL, NB], F32, name=f"ps_b{i}") for i in (1, 2)
            ]

            def h_expand_dve(dst, src):
                """dst (128, 28, 28); src (128, 5, 28) psum view."""
                nc.vector.tensor_copy(
                    out=dst[:, 2 : S - 2, :],
                    in_=src[:, 2:3, :].to_broadcast((BL, S - 4, S)),
                )
                nc.vector.tensor_copy(out=dst[:, 0:2, :], in_=src[:, 0:2, :])
                nc.vector.tensor_copy(
                    out=dst[:, S - 2 : S, :], in_=src[:, 3:5, :]
                )

            def h_expand_act(dst, src):
                """dst (128, 2, 28, 28); src (128, 2, 5, 28) psum view."""
                nc.scalar.copy(
                    dst[:, :, 2 : S - 2, :],
                    src[:, :, 2:3, :].to_broadcast((BL, 2, S - 4, S)),
                )
                nc.scalar.copy(dst[:, :, 0:2, :], src[:, :, 0:2, :])
                nc.scalar.copy(dst[:, :, S - 2 : S, :], src[:, :, 3:5, :])

            # ---- p0: minimal chain to the first output DMA ----
            # mm(cd2, 140 cols) -> 3 DVE copies -> teaser DMA (bcast src)
            ja = BL
            nc.tensor.matmul(
                ps_a0[:], xw_sb[:, :BL], xw_sb[:, ja : ja + NWC],
                start=True, stop=True,
            )
            h_expand_dve(
                ifirst0[:, 0],
                ps_a0[:].rearrange("b (ch w) -> b ch w", ch=NCLS),
            )
            iv = ifirst0[:].rearrange("b o h w -> b o (h w)")
            nc.sync.dma_start(
                out=out_v[:, 0, 2:6, :], in_=iv.to_broadcast((BL, 4, SS))
            )
            nc.tensor.matmul(
                ps_b0[:], xw_sb[:, :BL], xw_sb[:, ja + NWC : ja + NA],
                start=True, stop=True,
            )
            nc.tensor.matmul(
                ps_c0[:], xw_sb[:, :BL], xw_sb[:, ja + NA : ja + NA + NB],
                start=True, stop=True,
            )
            # replicate while the teaser streams; mid DMAs are PLAIN reads
            # of the 4-plane rep tile (plain 12544B units stream at ~430
            # GB/s; broadcast units lose 2.5-7%)
            nc.vector.tensor_copy(
                out=rep[0][:], in_=iv.to_broadcast((BL, 4, SS))
            )
            nc.sync.dma_start(out=out_v[:, 0, 6:10, :], in_=rep[0][:])
            nc.sync.dma_start(out=out_v[:, 0, 10:14, :], in_=rep[0][:])
            h_expand_act(
                edge[:, 0, 0],
                ps_b0[:].rearrange("b (cd ch w) -> b cd ch w", cd=2, ch=NCLS),
            )
            h_expand_act(
                edge[:, 1, 0],
                ps_c0[:].rearrange("b (cd ch w) -> b cd ch w", cd=2, ch=NCLS),
            )

            for p in (1, 2):
                ja = BL + p * (NA + NB)
                nc.tensor.matmul(
                    ps_a[p][:], xw_sb[:, :BL], xw_sb[:, ja : ja + NA],
                    start=True, stop=True,
                )
                nc.tensor.matmul(
                    ps_b[p][:], xw_sb[:, :BL], xw_sb[:, ja + NA : ja + NA + NB],
                    start=True, stop=True,
                )
                pa = ps_a[p][:].rearrange("b (cd ch w) -> b cd ch w", cd=3, ch=NCLS)
                pb = ps_b[p][:].rearrange("b (cd ch w) -> b cd ch w", cd=2, ch=NCLS)

                # expand into rep slot 0, then replicate to slots 1:4
                rp = rep[p][:].rearrange("b s (h w) -> b s h w", h=S)
                h_expand_dve(rp[:, 0], pa[:, 0])
                nc.vector.tensor_copy(
                    out=rep[p][:, 1:4],
                    in_=rep[p][:, 0:1].to_broadcast((BL, 3, SS)),
                )
                nc.sync.dma_start(out=out_v[:, p, 2:6, :], in_=rep[p][:])
                nc.sync.dma_start(out=out_v[:, p, 6:10, :], in_=rep[p][:])
                nc.sync.dma_start(out=out_v[:, p, 10:14, :], in_=rep[p][:])
                # d-edge planes on ACT
                h_expand_act(edge[:, 0, p], pa[:, 1:3])
                h_expand_act(edge[:, 1, p], pb[:, 0:2])

            # two merged edge DMAs: (b, 3p, 2d, 784) regions are affine in
            # the output; source reads are contiguous 18816B per partition
            ev = edge[:].rearrange("b e p s h w -> b e p s (h w)")
            nc.sync.dma_start(out=out_v[:, :, 0:2, :], in_=ev[:, 0])
            nc.sync.dma_start(out=out_v[:, :, F - 2 : F, :], in_=ev[:, 1])
    _split_multi_waits(nc)
    return nc


_CACHE = {}


def kernel(x, w1, b1, w2, b2):
    x = np.asarray(x, np.float32)
    w_aug = _fold_weights(
        np.asarray(w1, np.float64),
        np.asarray(b1, np.float64),
        np.asarray(w2, np.float64),
        np.asarray(b2, np.float64),
    )
    if "nc" not in _CACHE:
        _CACHE["nc"] = _build_bass()
    nc = _CACHE["nc"]

    # shard batch across cores; packed (4, 128+2100) bf16: x_aug^T | W
    in_maps = []
    for i in range(N_CORES):
        xs = x[i * BL : (i + 1) * BL]  # (128, 3)
        xa = np.concatenate([xs, np.ones((BL, 1), np.float32)], axis=1)  # (128,4)
        packed = np.concatenate(
            [xa.T.astype(ml_dtypes.bfloat16), w_aug], axis=1
        )
        in_maps.append({"xw": np.ascontiguousarray(packed)})
    res = run_bass_kernel_spmd(nc, in_maps, core_ids=list(range(N_CORES)))
    _CACHE["last_results"] = res  # exec_time_ns etc. when BASS_TRACE=1
    return np.concatenate([r["out"] for r in res.results], axis=0)
